# revision 1
# baseline (speedup 1.0000x reference)
"""Modulated deformable conv (DCNv2) Bass kernel for Trainium2, 8 NeuronCores.

Sharding: data-parallel over batch x row-halves; core i handles sample i//2,
output rows 64*(i%2) .. 64*(i%2)+63. No cross-core communication.

Per-core pipeline:
  B. PE: offset/mask conv (128ch -> 27ch, 3x3) as 9 shifted matmuls, PSUM acc.
  C. PE: transpose om to position-on-partition (natural + "wrapped" copy whose
     partition/free layout matches dma_gather's 16-partition index wrapping).
  D. DVE: fractional bilinear weights with mask/out-of-bounds validity folded
     in; gather indices (floor via exact mod-1 trick, all in a +4-shifted
     domain so values stay positive).
  E. PE: tiny transposes fold indices to the (16, n/16) wrapped layout; DVE
     casts to int16; index block replicated to all 128 partitions.
  F. SWDGE dma_gather: 512B tokens (two x-adjacent 64ch pixel vectors) from a
     duplicated-pair image in HBM -> position-on-partition tiles.
  G. DVE: weighted corner combine -> samp (per tap).
  H. PE: transpose samp to channel-on-partition; main conv accumulates all 9
     taps into PSUM; bias added during the PSUM->SBUF copy (ACT).
"""
import sys

for _p in ("/opt/trn_rl_repo", "/root/.axon_site/_ro/trn_rl_repo"):
    if _p not in sys.path:
        sys.path.append(_p)

import numpy as np

import concourse.bacc as bacc
import concourse.bass as bass
import concourse.mybir as mybir
import concourse.tile as tile
from concourse.masks import make_identity

F32 = mybir.dt.float32
I16 = mybir.dt.int16
ALU = mybir.AluOpType
ACTF = mybir.ActivationFunctionType

B, C, H, W = 4, 64, 128, 128
O, K2 = 64, 9
NCORES = 8
R = H // 2             # output rows per core
WP = W + 2             # padded image columns (left + right zero col)
NQ = 4                 # quarters of the per-core position space
GQ = 16                # row-chunks (=output rows) per quarter
NIDX = GQ * 128        # gather indices per dma_gather call
SH = 4.0               # +4 domain shift so floor domain is positive


def rr(t, spec, **kw):
    return t[:].rearrange(spec, **kw)


def build_program():
    nc = bacc.Bacc("TRN2")
    d_x2 = nc.dram_tensor("x2", [128, 66 * 130], F32, kind="ExternalInput")
    d_img = nc.dram_tensor("imgpairs", [H * WP, 128], F32, kind="ExternalInput")
    d_base = nc.dram_tensor("base", [128, 64 * 18], F32, kind="ExternalInput")
    d_basew = nc.dram_tensor("basew", [128, 16 * 4 * 18], F32, kind="ExternalInput")
    d_womt = nc.dram_tensor("womt", [128, 9 * 27], F32, kind="ExternalInput")
    d_bomt = nc.dram_tensor("bomt", [27, 1], F32, kind="ExternalInput")
    d_wmain = nc.dram_tensor("wmain", [64, 9 * 64], F32, kind="ExternalInput")
    d_biast = nc.dram_tensor("biast", [64, 1], F32, kind="ExternalInput")
    d_out = nc.dram_tensor("out", [64, R * W], F32, kind="ExternalOutput")

    with tile.TileContext(nc) as tc:
        with (
            tc.tile_pool(name="consts", bufs=1) as cpool,
        ):
            ident = cpool.tile([128, 128], F32)
            make_identity(nc, ident[:])
            womt = cpool.tile([128, 9 * 27], F32)
            bomt = cpool.tile([27, 1], F32)
            base = cpool.tile([128, 64 * 18], F32)
            basew = cpool.tile([128, 16 * 4 * 18], F32)
            wmain = cpool.tile([64, 9 * 64], F32)
            biast = cpool.tile([64, 1], F32)
            for sb, dr in ((womt, d_womt), (bomt, d_bomt), (base, d_base),
                           (basew, d_basew), (wmain, d_wmain), (biast, d_biast)):
                nc.sync.dma_start(sb[:], dr[:])

            wtop = cpool.tile([128, 64 * 18], F32)
            wbot = cpool.tile([128, 64 * 18], F32)
            idxw = cpool.tile([128, K2 * 2 * 4 * 128], I16)

            with (
                tc.tile_pool(name="mid", bufs=1) as midpool,
                tc.tile_pool(name="tmp", bufs=1) as tpool,
                tc.tile_pool(name="pso", bufs=2, space="PSUM") as ps_om,
                tc.tile_pool(name="pstp", bufs=2, space="PSUM") as ps_tp,
            ):
                omT = midpool.tile([128, 64 * 27], F32)
                omTw = midpool.tile([128, 16 * 4 * 18], F32)
                om = midpool.tile([27, R * W], F32)
                omTv = rr(omT, "p (g s) -> p g s", s=27)
                omTwv = rr(omTw, "p (q c s) -> p q c s", q=16, c=4)
                # ---- B: om conv ----
                x2 = midpool.tile([128, 66 * 130], F32)
                nc.sync.dma_start(x2[:], d_x2[:])
                x2v = rr(x2, "p (r c) -> p r c", c=130)
                for nt in range(16):
                    ps = ps_om.tile([27, 512], F32)
                    for d in range(9):
                        dy, dx = d // 3, d % 3
                        rhs = x2v[:, nt * 4 + dy: nt * 4 + dy + 4, dx: dx + 128]
                        nc.tensor.matmul(ps[:], lhsT=womt[:, d * 27:(d + 1) * 27],
                                         rhs=rhs, start=(d == 0), stop=(d == 8))
                    nc.scalar.activation(om[:, nt * 512:(nt + 1) * 512], ps[:],
                                         ACTF.Identity, bias=bomt[:, 0:1])

                # ---- C: omT natural ----
                for g in range(64):
                    pst = ps_tp.tile([128, 27], F32, tag="pst")
                    nc.tensor.transpose(pst[:], om[:, g * 128:(g + 1) * 128],
                                        ident[:27, :27])
                    nc.scalar.copy(omTv[:, g, :], pst[:])
                nc.scalar.activation(omTv[:, :, 18:27], omTv[:, :, 18:27],
                                     ACTF.Sigmoid)

                # ---- C2: omTw wrapped ----
                omv16 = rr(om, "s (a b) -> s a b", b=16)
                for q in range(16):
                    for cq in range(4):
                        src = omv16[0:18, 128 * cq:128 * cq + 128, q]
                        pw = ps_tp.tile([128, 18], F32, tag="pw")
                        nc.tensor.transpose(pw[:], src, ident[:18, :18])
                        nc.scalar.copy(omTwv[:, q, cq, :], pw[:])

                # ---- D: natural pipeline -> wtop/wbot corner-weight tiles ----
                basev = rr(base, "p (g s) -> p g s", s=18)

                def v18(t):
                    return rr(t, "p (g s) -> p g s", s=18)

                def v9(t):
                    return rr(t, "p (g s) -> p g s", s=9)

                pypx = tpool.tile([128, 64 * 18], F32)
                nc.vector.tensor_add(out=v18(pypx), in0=omTv[:, :, 0:18],
                                     in1=basev)
                ri32 = tpool.tile([128, 64 * 18], mybir.dt.int32)
                nc.vector.tensor_copy(ri32[:], pypx[:])
                rf32 = tpool.tile([128, 64 * 18], F32)
                nc.vector.tensor_copy(rf32[:], ri32[:])
                gt_ = tpool.tile([128, 64 * 18], F32, tag="ri32")
                nc.vector.tensor_tensor(out=gt_[:], in0=rf32[:], in1=pypx[:],
                                        op=ALU.is_gt)
                flor = tpool.tile([128, 64 * 18], F32)
                nc.vector.tensor_tensor(out=flor[:], in0=rf32[:], in1=gt_[:],
                                        op=ALU.subtract)
                frac = tpool.tile([128, 64 * 18], F32)
                nc.vector.tensor_tensor(out=frac[:], in0=pypx[:], in1=flor[:],
                                        op=ALU.subtract)
                f0c = tpool.tile([128, 64 * 18], F32)
                nc.vector.tensor_scalar(f0c[:], flor[:], SH, 127.0 + SH,
                                        ALU.max, ALU.min)
                v0 = tpool.tile([128, 64 * 18], F32)
                nc.vector.tensor_tensor(out=v0[:], in0=f0c[:], in1=flor[:],
                                        op=ALU.is_equal)
                f1 = tpool.tile([128, 64 * 18], F32)
                nc.vector.tensor_scalar(f1[:], flor[:], 1.0, None, ALU.add)
                f1c = tpool.tile([128, 64 * 18], F32)
                nc.vector.tensor_scalar(f1c[:], f1[:], SH, 127.0 + SH,
                                        ALU.max, ALU.min)
                v1 = tpool.tile([128, 64 * 18], F32)
                nc.vector.tensor_tensor(out=v1[:], in0=f1c[:], in1=f1[:],
                                        op=ALU.is_equal)

                wy, wx = v18(frac)[:, :, 0:9], v18(frac)[:, :, 9:18]
                vy0, vx0 = v18(v0)[:, :, 0:9], v18(v0)[:, :, 9:18]
                vy1, vx1 = v18(v1)[:, :, 0:9], v18(v1)[:, :, 9:18]
                msk = omTv[:, :, 18:27]

                a0 = tpool.tile([128, 64 * 9], F32)
                a1 = tpool.tile([128, 64 * 9], F32)
                b0 = tpool.tile([128, 64 * 9], F32)
                b1 = tpool.tile([128, 64 * 9], F32)
                a0v, a1v, b0v, b1v = v9(a0), v9(a1), v9(b0), v9(b1)
                nc.vector.tensor_scalar(a0[:], wy, -1.0, 1.0, ALU.mult, ALU.add)
                nc.vector.tensor_tensor(out=a0v, in0=a0v, in1=vy0, op=ALU.mult)
                nc.vector.tensor_tensor(out=a0v, in0=a0v, in1=msk, op=ALU.mult)
                nc.vector.tensor_tensor(out=a1v, in0=wy, in1=vy1, op=ALU.mult)
                nc.vector.tensor_tensor(out=a1v, in0=a1v, in1=msk, op=ALU.mult)
                nc.vector.tensor_scalar(b0[:], wx, -1.0, 1.0, ALU.mult, ALU.add)
                nc.vector.tensor_tensor(out=b0v, in0=b0v, in1=vx0, op=ALU.mult)
                nc.vector.tensor_tensor(out=b1v, in0=wx, in1=vx1, op=ALU.mult)

                # wtop slots s=k -> w00, s=9+k -> w01; wbot: w10 / w11
                nc.vector.tensor_tensor(out=v18(wtop)[:, :, 0:9], in0=a0v,
                                        in1=b0v, op=ALU.mult)
                nc.vector.tensor_tensor(out=v18(wtop)[:, :, 9:18], in0=a0v,
                                        in1=b1v, op=ALU.mult)
                nc.vector.tensor_tensor(out=v18(wbot)[:, :, 0:9], in0=a1v,
                                        in1=b0v, op=ALU.mult)
                nc.vector.tensor_tensor(out=v18(wbot)[:, :, 9:18], in0=a1v,
                                        in1=b1v, op=ALU.mult)

                # ---- D2: wrapped pipeline -> gather indices ----
                def w18(t):
                    return rr(t, "p (q c s) -> p q c s", q=16, c=4)

                def w9t(t):
                    return rr(t, "p (q c s) -> p q c s", q=16, c=4, s=9)

                pypw = tpool.tile([128, 16 * 4 * 18], F32)
                nc.vector.tensor_add(out=pypw[:], in0=omTw[:], in1=basew[:])
                ri32w = tpool.tile([128, 16 * 4 * 18], mybir.dt.int32)
                nc.vector.tensor_copy(ri32w[:], pypw[:])
                rf32w = tpool.tile([128, 16 * 4 * 18], F32)
                nc.vector.tensor_copy(rf32w[:], ri32w[:])
                gtw_ = tpool.tile([128, 16 * 4 * 18], F32, tag="ri32w")
                nc.vector.tensor_tensor(out=gtw_[:], in0=rf32w[:], in1=pypw[:],
                                        op=ALU.is_gt)
                florw = tpool.tile([128, 16 * 4 * 18], F32)
                nc.vector.tensor_tensor(out=florw[:], in0=rf32w[:], in1=gtw_[:],
                                        op=ALU.subtract)
                fy0w = tpool.tile([128, 16 * 4 * 9], F32)
                nc.vector.tensor_scalar(w9t(fy0w), w18(florw)[:, :, :, 0:9],
                                        SH, 127.0 + SH, ALU.max, ALU.min)
                gxw = tpool.tile([128, 16 * 4 * 9], F32)
                nc.vector.tensor_scalar(w9t(gxw), w18(florw)[:, :, :, 9:18],
                                        SH - 1.0, 127.0 + SH, ALU.max, ALU.min)
                # bottom row: clamp(flor+1, 4, 131) == clamp(flor, 3, 130) + 1
                fy1w = tpool.tile([128, 16 * 4 * 9], F32)
                nc.vector.tensor_scalar(w9t(fy1w), w18(florw)[:, :, :, 0:9],
                                        SH - 1.0, 126.0 + SH,
                                        ALU.max, ALU.min)
                idxt = tpool.tile([128, 16 * 4 * 9], F32)
                idxb = tpool.tile([128, 16 * 4 * 9], F32)
                KOFF = -(SH * WP + (SH - 1.0))  # -(4*130 + 3) = -523
                nc.vector.tensor_scalar(idxt[:], fy0w[:], float(WP), KOFF,
                                        ALU.mult, ALU.add)
                nc.vector.tensor_tensor(out=idxt[:], in0=idxt[:], in1=gxw[:],
                                        op=ALU.add)
                nc.vector.tensor_scalar(idxb[:], fy1w[:], float(WP),
                                        KOFF + float(WP), ALU.mult, ALU.add)
                nc.vector.tensor_tensor(out=idxb[:], in0=idxb[:], in1=gxw[:],
                                        op=ALU.add)

                # ---- E: fold indices to wrapped int16 layout ----
                for k in range(K2):
                    for tb in range(2):
                        srcT = w9t(idxt if tb == 0 else idxb)
                        for cq in range(4):
                            pv = ps_tp.tile([16, 128], F32, tag="pidx")
                            nc.tensor.transpose(pv[:], srcT[:, :, cq, k],
                                                ident[:, :])
                            off = ((k * 2 + tb) * 4 + cq) * 128
                            nc.vector.tensor_copy(idxw[0:16, off:off + 128],
                                                  pv[:])
                for g in range(1, 8):
                    nc.sync.dma_start(idxw[16 * g:16 * (g + 1), :],
                                      idxw[0:16, :])

            # ---- F/G/H: gather, combine, transpose, main conv ----
            wtopv = rr(wtop, "p (g s) -> p g s", s=18)
            wbotv = rr(wbot, "p (g s) -> p g s", s=18)
            with (
                tc.tile_pool(name="gat", bufs=2) as gpool,
                tc.tile_pool(name="comb", bufs=2) as mpool,
                tc.tile_pool(name="pstx", bufs=2, space="PSUM") as ps_tx,
                tc.tile_pool(name="psmain", bufs=1, space="PSUM") as ps_main,
            ):
                out_sb = gpool.tile([64, R * W], F32, tag="out_sb")
                nidx_reg = nc.gpsimd.to_reg(NIDX)
                for cq in range(NQ):
                    ops = ps_main.tile([64, 2048], F32)
                    for k in range(K2):
                        gt = gpool.tile([128, GQ * 128], F32, tag="gt")
                        gb = gpool.tile([128, GQ * 128], F32, tag="gb")
                        ot = ((k * 2 + 0) * 4 + cq) * 128
                        ob = ((k * 2 + 1) * 4 + cq) * 128
                        nc.gpsimd.dma_gather(rr(gt, "p (g c) -> p g c", g=GQ),
                                             d_img[:], idxw[:, ot:ot + 128],
                                             NIDX, nidx_reg, 128,
                                             single_packet=False)
                        nc.gpsimd.dma_gather(rr(gb, "p (g c) -> p g c", g=GQ),
                                             d_img[:], idxw[:, ob:ob + 128],
                                             NIDX, nidx_reg, 128,
                                             single_packet=False)
                        gtv = rr(gt, "p (g j c) -> p g j c", g=GQ, j=2)
                        gbv = rr(gb, "p (g j c) -> p g j c", g=GQ, j=2)
                        wt_k = wtopv[:, 16 * cq:16 * cq + 16, k::9]
                        wb_k = wbotv[:, 16 * cq:16 * cq + 16, k::9]
                        tt = mpool.tile([128, GQ * 2 * 64], F32, tag="tt")
                        tb_ = mpool.tile([128, GQ * 2 * 64], F32, tag="tb")
                        ttv = rr(tt, "p (g j c) -> p g j c", g=GQ, j=2)
                        tbv = rr(tb_, "p (g j c) -> p g j c", g=GQ, j=2)
                        nc.vector.tensor_tensor(
                            out=ttv, in0=gtv,
                            in1=wt_k[:, :, :, None].to_broadcast(
                                [128, GQ, 2, 64]),
                            op=ALU.mult)
                        nc.vector.tensor_tensor(
                            out=tbv, in0=gbv,
                            in1=wb_k[:, :, :, None].to_broadcast(
                                [128, GQ, 2, 64]),
                            op=ALU.mult)
                        samp = mpool.tile([128, GQ * 64], F32, tag="samp")
                        smpb = mpool.tile([128, GQ * 64], F32, tag="smpb")
                        sampv = rr(samp, "p (g c) -> p g c", g=GQ)
                        smpbv = rr(smpb, "p (g c) -> p g c", g=GQ)
                        nc.vector.tensor_add(out=sampv, in0=ttv[:, :, 0, :],
                                             in1=ttv[:, :, 1, :])
                        nc.vector.tensor_add(out=smpbv, in0=tbv[:, :, 0, :],
                                             in1=tbv[:, :, 1, :])
                        nc.vector.tensor_add(out=sampv, in0=sampv, in1=smpbv)
                        sampT = mpool.tile([64, GQ * 128], F32, tag="sampT")
                        for half in range(2):
                            px = ps_tx.tile([64, 1024], F32, tag="px")
                            for j8 in range(8):
                                g16 = half * 8 + j8
                                nc.tensor.transpose(
                                    px[:, j8 * 128:(j8 + 1) * 128],
                                    sampv[:, g16, :], ident[:, :])
                            nc.scalar.copy(
                                sampT[:, half * 1024:(half + 1) * 1024], px[:])
                        for gb4 in range(4):
                            nc.tensor.matmul(
                                ops[:, gb4 * 512:(gb4 + 1) * 512],
                                lhsT=wmain[:, k * 64:(k + 1) * 64],
                                rhs=sampT[:, gb4 * 512:(gb4 + 1) * 512],
                                start=(k == 0), stop=(k == K2 - 1))
                    nc.scalar.activation(
                        out_sb[:, cq * 2048:(cq + 1) * 2048], ops[:],
                        ACTF.Identity, bias=biast[:, 0:1])
            nc.sync.dma_start(d_out[:], out_sb[:])
    nc.compile()
    return nc


def _prep_core(inputs, core):
    b, r = core // 2, core % 2
    r0 = r * R
    keyt = np.ascontiguousarray(inputs["input_keyt"][b], np.float32)
    inter = np.ascontiguousarray(inputs["inter"][b], np.float32)
    weight = np.asarray(inputs["weight"], np.float32)
    bias = np.asarray(inputs["bias"], np.float32)
    w_om = np.asarray(inputs["w_om"], np.float32)
    b_om = np.asarray(inputs["b_om"], np.float32)

    x2full = np.concatenate([keyt, inter], axis=0)          # (128, 128, 128)
    x2c = np.zeros((128, 66, 130), np.float32)
    lo, hi = max(0, r0 - 1), min(H, r0 + R + 1)
    x2c[:, lo - (r0 - 1):hi - (r0 - 1), 1:129] = x2full[:, lo:hi, :]
    x2 = x2c.reshape(128, -1)

    im = keyt.transpose(1, 2, 0)                            # (H, W, C)
    flat = np.zeros((H, WP, C), np.float32)
    flat[:, 1:W + 1, :] = im
    flat = flat.reshape(H * WP, C)
    flat2 = np.vstack([flat, np.zeros((1, C), np.float32)])
    imgpairs = np.hstack([flat2[:-1], flat2[1:]])           # (H*WP, 128)

    ky = (np.arange(K2) // 3).astype(np.float32)
    kx = (np.arange(K2) % 3).astype(np.float32)
    p_ = np.arange(128, dtype=np.float32)
    g_ = np.arange(64, dtype=np.float32)
    base = np.zeros((128, 64, 18), np.float32)
    base[:, :, 0:9] = (r0 + g_[None, :, None]) - 1 + ky[None, None, :] + SH
    base[:, :, 9:18] = p_[:, None, None] - 1 + kx[None, None, :] + SH

    j_ = np.arange(128)[:, None, None]
    q_ = np.arange(16)[None, :, None]
    c_ = np.arange(4)[None, None, :]
    pg = 16 * (128 * c_ + j_) + q_                          # (128,16,4)
    hl, wl = pg // 128, pg % 128
    basew = np.zeros((128, 16, 4, 18), np.float32)
    basew[:, :, :, 0:9] = (r0 + hl)[..., None] - 1 + ky + SH
    basew[:, :, :, 9:18] = wl[..., None] - 1 + kx + SH

    womt = np.zeros((128, 9, 27), np.float32)
    for d in range(9):
        womt[:, d, :] = w_om[:, :, d // 3, d % 3].T
    W9 = weight.reshape(O, C, K2)
    wmain = np.zeros((64, 9, 64), np.float32)
    for k in range(K2):
        wmain[:, k, :] = W9[:, :, k].T

    return {
        "x2": x2,
        "imgpairs": imgpairs,
        "base": base.reshape(128, -1),
        "basew": basew.reshape(128, -1),
        "womt": womt.reshape(128, -1),
        "bomt": b_om.reshape(27, 1).astype(np.float32),
        "wmain": wmain.reshape(64, -1),
        "biast": bias.reshape(64, 1).astype(np.float32),
    }


_PROG = None


def kernel(**inputs) -> np.ndarray:
    global _PROG
    from concourse.bass_utils import run_bass_kernel_spmd
    if _PROG is None:
        _PROG = build_program()
    in_maps = [_prep_core(inputs, i) for i in range(NCORES)]
    res = run_bass_kernel_spmd(_PROG, in_maps, core_ids=list(range(NCORES)))
    out = np.zeros((B, O, H, W), np.float32)
    for i in range(NCORES):
        b, r = i // 2, i % 2
        out[b][:, r * R:(r + 1) * R, :] = res.results[i]["out"].reshape(O, R, W)
    return out



# revision 12
# speedup vs baseline: 1.9159x; 1.9159x over previous
"""Modulated deformable conv (DCNv2) Bass kernel for Trainium2, 8 NeuronCores.

Sharding: data-parallel over batch x row-halves; core i handles sample i//2,
output rows 64*(i%2) .. 64*(i%2)+63. No cross-core communication.

v2: bf16 datapath + SWDGE gather of 2x2-patch tokens (512B, halves the
gpsimd descriptor-generation load and HBM bytes vs pair tokens).

Per-core pipeline:
  B. PE: offset/mask conv (128ch -> 27ch, 3x3) as 9 shifted matmuls (bf16),
     PSUM acc, bias via ACT copy.
  C. PE: transpose om to position-on-partition (omT[col, row, 27], fp32).
  D. DVE: fractional bilinear weights with mask + out-of-bounds validity
     folded into 4 corner weights (duplicated x2 for packed-mode reads,
     bf16), plus patch-anchor gather indices (int32).
  F. Indirect DMA gather (hardware DGE): 512B tokens = 2x2 pixel patch
     x 64ch bf16 from a host-prebuilt patch table in HBM; one call per
     (quarter, tap-pair) = 20 calls, 2048/4096 tokens each.
  G. DVE: corner-weight multiply (packed 2x mode) + 2 pair adds -> samp.
  H. PE: paired-tap transposes ([128,128] bf16) + main conv with 128-deep
     contraction (2 taps x 64ch); bias added during PSUM->SBUF copy (ACT).
"""
import sys

for _p in ("/opt/trn_rl_repo", "/root/.axon_site/_ro/trn_rl_repo"):
    if _p not in sys.path:
        sys.path.append(_p)

import numpy as np
import ml_dtypes

import concourse.bacc as bacc
import concourse.bass as bass
import concourse.mybir as mybir
import concourse.tile as tile
from concourse.masks import make_identity

F32 = mybir.dt.float32
BF16 = mybir.dt.bfloat16
I32 = mybir.dt.int32
ALU = mybir.AluOpType
ACTF = mybir.ActivationFunctionType
BF = ml_dtypes.bfloat16

B, C, H, W = 4, 64, 128, 128
O, K2 = 64, 9
NCORES = 8
R = H // 2             # output rows per core
PW = 130               # patch-table width (anchors -1..128)
NQ = 4                 # quarters of the per-core position space
GQ = 16                # row-chunks (=output rows) per quarter
SH = 4.0               # +4 domain shift so floor domain is positive
KOFF = -(3.0 * PW + 3.0)   # anchor idx = (y0s-3)*130 + (x0s-3)


def rr(t, spec, **kw):
    return t[:].rearrange(spec, **kw)


def build_program():
    nc = bacc.Bacc("TRN2")
    d_x2 = nc.dram_tensor("x2", [128, 66 * PW], BF16, kind="ExternalInput")
    d_patch = nc.dram_tensor("patch", [PW * PW, 256], BF16, kind="ExternalInput")
    d_base = nc.dram_tensor("base", [128, 64 * 18], F32, kind="ExternalInput")
    d_basew = nc.dram_tensor("basew", [128, 16 * 4 * 18], F32, kind="ExternalInput")
    d_womt = nc.dram_tensor("womt", [128, 9 * 27], BF16, kind="ExternalInput")
    d_bomt = nc.dram_tensor("bomt", [27, 1], F32, kind="ExternalInput")
    d_wm2 = nc.dram_tensor("wm2", [128, 4 * 64], BF16, kind="ExternalInput")
    d_wms = nc.dram_tensor("wms", [64, 64], BF16, kind="ExternalInput")
    d_biast = nc.dram_tensor("biast", [64, 1], F32, kind="ExternalInput")
    d_out = nc.dram_tensor("out", [64, R * W], F32, kind="ExternalOutput")

    with tile.TileContext(nc) as tc:
        with (
            tc.tile_pool(name="consts", bufs=1) as cpool,
        ):
            identb = cpool.tile([128, 128], BF16)
            make_identity(nc, identb[:])
            identf = cpool.tile([128, 128], F32)
            make_identity(nc, identf[:])
            womt = cpool.tile([128, 9 * 27], BF16)
            bomt = cpool.tile([27, 1], F32)
            base = cpool.tile([128, 64 * 18], F32)
            basew = cpool.tile([128, 16 * 4 * 18], F32)
            wm2 = cpool.tile([128, 4 * 64], BF16)
            wms = cpool.tile([64, 64], BF16)
            biast = cpool.tile([64, 1], F32)
            for sb, dr in ((womt, d_womt), (bomt, d_bomt), (base, d_base),
                           (basew, d_basew), (wm2, d_wm2), (wms, d_wms),
                           (biast, d_biast)):
                nc.sync.dma_start(sb[:], dr[:])

            # corner weights [p, g64, k9, j4, dup2] bf16 + gather indices
            wdup = cpool.tile([128, 64 * 9 * 4 * 2], BF16)
            idxw = cpool.tile([128, 9 * 4 * 128], mybir.dt.int16)

            with (
                tc.tile_pool(name="mid", bufs=1) as midpool,
                tc.tile_pool(name="tmp", bufs=1) as tpool,
                tc.tile_pool(name="pso", bufs=2, space="PSUM") as ps_om,
                tc.tile_pool(name="pstp", bufs=2, space="PSUM") as ps_tp,
            ):
                omT = midpool.tile([128, 64 * 27], F32)
                om = midpool.tile([27, R * W], F32)
                omTv = rr(omT, "p (g s) -> p g s", s=27)
                # ---- B: om conv ----
                x2 = midpool.tile([128, 66 * PW], BF16)
                nc.sync.dma_start(x2[:], d_x2[:])
                x2v = rr(x2, "p (r c) -> p r c", c=PW)
                for nt in range(16):
                    ps = ps_om.tile([27, 512], F32)
                    for d in range(9):
                        dy, dx = d // 3, d % 3
                        rhs = x2v[:, nt * 4 + dy: nt * 4 + dy + 4, dx: dx + 128]
                        nc.tensor.matmul(ps[:], lhsT=womt[:, d * 27:(d + 1) * 27],
                                         rhs=rhs, start=(d == 0), stop=(d == 8))
                    nc.scalar.activation(om[:, nt * 512:(nt + 1) * 512], ps[:],
                                         ACTF.Identity, bias=bomt[:, 0:1])

                # ---- C: omT natural (position-on-partition) ----
                for g in range(64):
                    pst = ps_tp.tile([128, 27], F32, tag="pst")
                    nc.tensor.transpose(pst[:], om[:, g * 128:(g + 1) * 128],
                                        identf[:27, :27])
                    nc.scalar.copy(omTv[:, g, :], pst[:])
                nc.scalar.activation(omTv[:, :, 18:27], omTv[:, :, 18:27],
                                     ACTF.Sigmoid)

                # ---- D: bilinear corner weights + gather indices ----
                basev = rr(base, "p (g s) -> p g s", s=18)

                def v18(t):
                    return rr(t, "p (g s) -> p g s", s=18)

                def v9(t):
                    return rr(t, "p (g s) -> p g s", s=9)

                pypx = tpool.tile([128, 64 * 18], F32)
                nc.vector.tensor_add(out=v18(pypx), in0=omTv[:, :, 0:18],
                                     in1=basev)
                ri32 = tpool.tile([128, 64 * 18], mybir.dt.int32)
                nc.vector.tensor_copy(ri32[:], pypx[:])
                rf32 = tpool.tile([128, 64 * 18], F32)
                nc.vector.tensor_copy(rf32[:], ri32[:])
                gt_ = tpool.tile([128, 64 * 18], F32, tag="ri32")
                nc.vector.tensor_tensor(out=gt_[:], in0=rf32[:], in1=pypx[:],
                                        op=ALU.is_gt)
                flor = tpool.tile([128, 64 * 18], F32)
                nc.vector.tensor_tensor(out=flor[:], in0=rf32[:], in1=gt_[:],
                                        op=ALU.subtract)
                frac = tpool.tile([128, 64 * 18], F32)
                nc.vector.tensor_tensor(out=frac[:], in0=pypx[:], in1=flor[:],
                                        op=ALU.subtract)
                f0c = tpool.tile([128, 64 * 18], F32)
                nc.vector.tensor_scalar(f0c[:], flor[:], SH, 127.0 + SH,
                                        ALU.max, ALU.min)
                v0 = tpool.tile([128, 64 * 18], F32)
                nc.vector.tensor_tensor(out=v0[:], in0=f0c[:], in1=flor[:],
                                        op=ALU.is_equal)
                f1 = tpool.tile([128, 64 * 18], F32)
                nc.vector.tensor_scalar(f1[:], flor[:], 1.0, None, ALU.add)
                f1c = tpool.tile([128, 64 * 18], F32)
                nc.vector.tensor_scalar(f1c[:], f1[:], SH, 127.0 + SH,
                                        ALU.max, ALU.min)
                v1 = tpool.tile([128, 64 * 18], F32)
                nc.vector.tensor_tensor(out=v1[:], in0=f1c[:], in1=f1[:],
                                        op=ALU.is_equal)

                wy, wx = v18(frac)[:, :, 0:9], v18(frac)[:, :, 9:18]
                vy0, vx0 = v18(v0)[:, :, 0:9], v18(v0)[:, :, 9:18]
                vy1, vx1 = v18(v1)[:, :, 0:9], v18(v1)[:, :, 9:18]
                msk = omTv[:, :, 18:27]

                a0 = tpool.tile([128, 64 * 9], F32)
                a1 = tpool.tile([128, 64 * 9], F32)
                b0 = tpool.tile([128, 64 * 9], F32)
                b1 = tpool.tile([128, 64 * 9], F32)
                a0v, a1v, b0v, b1v = v9(a0), v9(a1), v9(b0), v9(b1)
                nc.vector.tensor_scalar(a0[:], wy, -1.0, 1.0, ALU.mult, ALU.add)
                nc.vector.tensor_tensor(out=a0v, in0=a0v, in1=vy0, op=ALU.mult)
                nc.vector.tensor_tensor(out=a0v, in0=a0v, in1=msk, op=ALU.mult)
                nc.vector.tensor_tensor(out=a1v, in0=wy, in1=vy1, op=ALU.mult)
                nc.vector.tensor_tensor(out=a1v, in0=a1v, in1=msk, op=ALU.mult)
                nc.vector.tensor_scalar(b0[:], wx, -1.0, 1.0, ALU.mult, ALU.add)
                nc.vector.tensor_tensor(out=b0v, in0=b0v, in1=vx0, op=ALU.mult)
                nc.vector.tensor_tensor(out=b1v, in0=wx, in1=vx1, op=ALU.mult)

                # corner weights, each duplicated x2 along the innermost dim
                wdv = rr(wdup, "p (g k j d) -> p g k j d", g=64, k=9, j=4)
                for j, (ya, xb) in enumerate(((a0v, b0v), (a0v, b1v),
                                              (a1v, b0v), (a1v, b1v))):
                    nc.vector.tensor_tensor(
                        out=wdv[:, :, :, j, :],
                        in0=ya[:, :, :, None].to_broadcast([128, 64, 9, 2]),
                        in1=xb[:, :, :, None].to_broadcast([128, 64, 9, 2]),
                        op=ALU.mult)

                # ---- C2: omTw wrapped (offsets in dma_gather's 16-wrap
                # position order; only needed for the gather indices) ----
                omTw = midpool.tile([128, 16 * 4 * 18], F32)
                omTwv = rr(omTw, "p (q c s) -> p q c s", q=16, c=4)
                omv16 = rr(om, "s (a b) -> s a b", b=16)
                for q in range(16):
                    for cq in range(4):
                        src = omv16[0:18, 128 * cq:128 * cq + 128, q]
                        pw = ps_tp.tile([128, 18], F32, tag="pw")
                        nc.tensor.transpose(pw[:], src, identf[:18, :18])
                        nc.scalar.copy(omTwv[:, q, cq, :], pw[:])

                # ---- D2: wrapped pipeline -> patch anchor indices ----
                def w18(t):
                    return rr(t, "p (q c s) -> p q c s", q=16, c=4)

                def w9t(t):
                    return rr(t, "p (q c s) -> p q c s", q=16, c=4, s=9)

                pypw = tpool.tile([128, 16 * 4 * 18], F32)
                nc.vector.tensor_add(out=pypw[:], in0=omTw[:], in1=basew[:])
                ri32w = tpool.tile([128, 16 * 4 * 18], mybir.dt.int32)
                nc.vector.tensor_copy(ri32w[:], pypw[:])
                rf32w = tpool.tile([128, 16 * 4 * 18], F32)
                nc.vector.tensor_copy(rf32w[:], ri32w[:])
                gtw_ = tpool.tile([128, 16 * 4 * 18], F32, tag="ri32w")
                nc.vector.tensor_tensor(out=gtw_[:], in0=rf32w[:], in1=pypw[:],
                                        op=ALU.is_gt)
                florw = tpool.tile([128, 16 * 4 * 18], F32)
                nc.vector.tensor_tensor(out=florw[:], in0=rf32w[:],
                                        in1=gtw_[:], op=ALU.subtract)
                # clamp anchors to [-1, 128] (+SH domain: [3, 132])
                fy0w = tpool.tile([128, 16 * 4 * 9], F32)
                nc.vector.tensor_scalar(w9t(fy0w), w18(florw)[:, :, :, 0:9],
                                        SH - 1.0, 128.0 + SH, ALU.max, ALU.min)
                gxw = tpool.tile([128, 16 * 4 * 9], F32)
                nc.vector.tensor_scalar(w9t(gxw), w18(florw)[:, :, :, 9:18],
                                        SH - 1.0, 128.0 + SH, ALU.max, ALU.min)
                idxt = tpool.tile([128, 16 * 4 * 9], F32)
                # anchor = (y0s-3)*130 + (x0s-3), y0s/x0s in +4 domain
                nc.vector.tensor_scalar(idxt[:], fy0w[:], float(PW), KOFF,
                                        ALU.mult, ALU.add)
                nc.vector.tensor_tensor(out=idxt[:], in0=idxt[:], in1=gxw[:],
                                        op=ALU.add)

                # ---- E: fold indices to wrapped int16 layout [cq][k][128]
                for k in range(K2):
                    srcT = w9t(idxt)
                    for cq in range(4):
                        pv = ps_tp.tile([16, 128], F32, tag="pidx")
                        nc.tensor.transpose(pv[:], srcT[:, :, cq, k],
                                            identf[:, :])
                        off = (cq * 9 + k) * 128
                        nc.vector.tensor_copy(idxw[0:16, off:off + 128],
                                              pv[:])
                for g in range(1, 8):
                    nc.sync.dma_start(idxw[16 * g:16 * (g + 1), :],
                                      idxw[0:16, :])

            # ---- F/G/H: gather, combine, transpose, main conv ----
            wdv2 = rr(wdup, "p (g k j d) -> p g k j d", g=64, k=9, j=4)
            with (
                tc.tile_pool(name="gat", bufs=2) as gpool,
                tc.tile_pool(name="comb", bufs=2) as mpool,
                tc.tile_pool(name="pstx", bufs=2, space="PSUM") as ps_tx,
                tc.tile_pool(name="psmain", bufs=1, space="PSUM") as ps_main,
            ):
                out_sb = gpool.tile([64, R * W], F32, tag="out_sb")
                reg2k = nc.gpsimd.to_reg(2048)
                reg4k = nc.gpsimd.to_reg(4096)
                for cq in range(NQ):
                    ops = ps_main.tile([64, 2048], F32)
                    for pr in range(5):
                        npair = 2 if pr < 4 else 1
                        nslot = GQ * npair
                        gt = gpool.tile([128, nslot * 256], BF16,
                                        tag=f"gt{npair}")
                        k0 = 2 * pr
                        ioff = (cq * 9 + k0) * 128
                        nc.gpsimd.dma_gather(
                            rr(gt, "p (i e) -> p i e", e=256),
                            d_patch[:],
                            idxw[:, ioff:ioff + 128 * npair],
                            2048 * npair,
                            reg4k if npair == 2 else reg2k,
                            256,
                            single_packet=False)
                        # combine per tap
                        if npair == 2:
                            samp2 = mpool.tile([128, GQ * 128], BF16,
                                               tag="s2")
                        else:
                            samp2 = mpool.tile([128, GQ * 64], BF16,
                                               tag="s2s")
                        for t in range(npair):
                            k = k0 + t
                            tt = mpool.tile([128, GQ * 256], BF16, tag="tt")
                            ttv = rr(tt, "p (g j c2 d) -> p g j c2 d",
                                     g=GQ, j=4, c2=32)
                            gtv = rr(gt, "p (i j c2 d) -> p i j c2 d",
                                     i=nslot, j=4, c2=32)
                            for j in range(4):
                                nc.vector.tensor_tensor(
                                    out=ttv[:, :, j, :, :],
                                    in0=gtv[:, t * GQ:(t + 1) * GQ, j, :, :],
                                    in1=wdv2[:, 16 * cq:16 * cq + 16, k, j,
                                             None, :].to_broadcast(
                                                 [128, GQ, 32, 2]),
                                    op=ALU.mult)
                            u = mpool.tile([128, GQ * 128], BF16, tag="u")
                            uv = rr(u, "p (g j c) -> p g j c", g=GQ, j=2)
                            ttj = rr(tt, "p (g j c) -> p g j c", g=GQ, j=4)
                            nc.vector.tensor_tensor(
                                out=uv[:], in0=ttj[:, :, 0:2, :],
                                in1=ttj[:, :, 2:4, :], op=ALU.add)
                            if npair == 2:
                                s2v = rr(samp2, "p (g w2 c) -> p g w2 c",
                                         g=GQ, w2=2)
                                outv = s2v[:, :, t, :]
                            else:
                                outv = rr(samp2, "p (g c) -> p g c", g=GQ)
                            nc.vector.tensor_tensor(
                                out=outv, in0=uv[:, :, 0, :],
                                in1=uv[:, :, 1, :], op=ALU.add)
                        # transpose to channel-on-partition
                        cw = 128 if npair == 2 else 64
                        sampT = mpool.tile([cw, GQ * 128], BF16, tag=f"sT{cw}")
                        for half in range(2):
                            px = ps_tx.tile([128, 1024], BF16, tag="px")
                            for j8 in range(8):
                                g16 = half * 8 + j8
                                nc.tensor.transpose(
                                    px[0:cw, j8 * 128:(j8 + 1) * 128],
                                    samp2[:, g16 * cw:(g16 + 1) * cw],
                                    identb[:, :])
                            nc.scalar.copy(
                                sampT[:, half * 1024:(half + 1) * 1024],
                                px[0:cw, :])
                        lhsT = wm2[:, pr * 64:(pr + 1) * 64] if npair == 2 \
                            else wms[:, :]
                        for gb4 in range(4):
                            nc.tensor.matmul(
                                ops[:, gb4 * 512:(gb4 + 1) * 512],
                                lhsT=lhsT,
                                rhs=sampT[:, gb4 * 512:(gb4 + 1) * 512],
                                start=(pr == 0), stop=(pr == 4))
                    nc.scalar.activation(
                        out_sb[:, cq * 2048:(cq + 1) * 2048], ops[:],
                        ACTF.Identity, bias=biast[:, 0:1])
            nc.sync.dma_start(d_out[:], out_sb[:])
    nc.compile()
    return nc


def _prep_core(inputs, core):
    b, r = core // 2, core % 2
    r0 = r * R
    keyt = np.ascontiguousarray(inputs["input_keyt"][b], np.float32)
    inter = np.ascontiguousarray(inputs["inter"][b], np.float32)
    weight = np.asarray(inputs["weight"], np.float32)
    bias = np.asarray(inputs["bias"], np.float32)
    w_om = np.asarray(inputs["w_om"], np.float32)
    b_om = np.asarray(inputs["b_om"], np.float32)

    x2full = np.concatenate([keyt, inter], axis=0)          # (128, 128, 128)
    x2c = np.zeros((128, 66, PW), np.float32)
    lo, hi = max(0, r0 - 1), min(H, r0 + R + 1)
    x2c[:, lo - (r0 - 1):hi - (r0 - 1), 1:129] = x2full[:, lo:hi, :]
    x2 = x2c.reshape(128, -1).astype(BF)

    # 2x2 patch tokens, anchors (y0, x0) in [-1, 128]^2, token [jy, jx, c]
    im = keyt.transpose(1, 2, 0)                            # (H, W, C)
    Z = np.zeros((H + 4, W + 4, C), np.float32)
    Z[2:H + 2, 2:W + 2] = im
    # anchor a=y0+1 in [0,130): rows y0+jy = a-1+jy -> Z[a+1+jy]
    P00 = Z[1:1 + PW, 1:1 + PW]
    P01 = Z[1:1 + PW, 2:2 + PW]
    P10 = Z[2:2 + PW, 1:1 + PW]
    P11 = Z[2:2 + PW, 2:2 + PW]
    patch = np.stack([P00, P01, P10, P11], axis=2)          # (130,130,4,C)
    patch = patch.reshape(PW * PW, 4 * C).astype(BF)

    ky = (np.arange(K2) // 3).astype(np.float32)
    kx = (np.arange(K2) % 3).astype(np.float32)
    p_ = np.arange(128, dtype=np.float32)
    g_ = np.arange(64, dtype=np.float32)
    base = np.zeros((128, 64, 18), np.float32)
    base[:, :, 0:9] = (r0 + g_[None, :, None]) - 1 + ky[None, None, :] + SH
    base[:, :, 9:18] = p_[:, None, None] - 1 + kx[None, None, :] + SH

    j_ = np.arange(128)[:, None, None]
    q_ = np.arange(16)[None, :, None]
    c_ = np.arange(4)[None, None, :]
    pg = 16 * (128 * c_ + j_) + q_                          # (128,16,4)
    hl, wl = pg // 128, pg % 128
    basew = np.zeros((128, 16, 4, 18), np.float32)
    basew[:, :, :, 0:9] = (r0 + hl)[..., None] - 1 + ky + SH
    basew[:, :, :, 9:18] = wl[..., None] - 1 + kx + SH

    womt = np.zeros((128, 9, 27), np.float32)
    for d in range(9):
        womt[:, d, :] = w_om[:, :, d // 3, d % 3].T
    W9 = weight.reshape(O, C, K2)
    wm2 = np.zeros((128, 4, 64), np.float32)
    for pr in range(4):
        for i in range(2):
            wm2[64 * i:64 * (i + 1), pr, :] = W9[:, :, 2 * pr + i].T
    wms = np.ascontiguousarray(W9[:, :, 8].T)

    return {
        "x2": x2,
        "patch": patch,
        "base": base.reshape(128, -1),
        "basew": basew.reshape(128, -1),
        "womt": womt.reshape(128, -1).astype(BF),
        "bomt": b_om.reshape(27, 1).astype(np.float32),
        "wm2": wm2.reshape(128, -1).astype(BF),
        "wms": wms.astype(BF),
        "biast": bias.reshape(64, 1).astype(np.float32),
    }


_PROG = None


def kernel(**inputs) -> np.ndarray:
    global _PROG
    from concourse.bass_utils import run_bass_kernel_spmd
    if _PROG is None:
        _PROG = build_program()
    in_maps = [_prep_core(inputs, i) for i in range(NCORES)]
    res = run_bass_kernel_spmd(_PROG, in_maps, core_ids=list(range(NCORES)))
    out = np.zeros((B, O, H, W), np.float32)
    for i in range(NCORES):
        b, r = i // 2, i % 2
        out[b][:, r * R:(r + 1) * R, :] = res.results[i]["out"].reshape(O, R, W)
    return out
